# revision 4
# baseline (speedup 1.0000x reference)
"""Trainium2 Bass kernel for nn_ModelNew_78847009620052 (dense_mlp).

Computes, for x [4096, 8192] and weight [8192, 8192]:
    out[b, 0] = 0.75 * sum_i x[b, i] * (sum_j weight[j, i])
(which equals 1.5 * sum(x @ W.T / 2, axis=1, keepdims=True)).

Sharding: column-shard the contraction dim IN=8192 into 8 chunks of 1024.
Core d receives x[:, d*1024:(d+1)*1024] and weight[:, d*1024:(d+1)*1024],
produces a partial [4096, 1]; host sums the 8 partials.

The kernel is DMA-bound (48 MiB of input per core; chip HBM delivers
~2.9 TB/s across the 8 cores, so ~370 GB/s/core when all queues stay
pressured). v2 layout changes vs the original:

  * Host-side repack to partition-major [128, T, 1024] for both inputs
    (row assignment is free for weight since rows are summed; for x the
    (p, i) layout matches the output store order). One DMA then moves a
    [128, G, 1024] group whose per-partition source bytes are G*4KiB
    CONTIGUOUS - large descriptors, few instructions, few semaphores.
  * Weight stream: 15x 2MiB + [2,1,1] tail groups (descending so the
    column-sum finishes ~2us after the last weight byte lands).
  * x stream: [1,1,2,4x6,2,1,1] - small leading groups so phase-2
    compute starts as soon as the weight stream ends, small trailing
    groups to keep the post-stream drain to one tile.
  * Everything issues on the sync-engine HWDGE queue in stream order;
    deep tile pools (5 x 2MiB per stream) keep the ring fed across
    buffer-recycle waits.

Per-core device algorithm:
  Phase 1: per weight group, tree-add the 2MiB group to one [128, 1024]
           tile on VectorE, then matmul with an all-ones [128, 128]
           stationary into PSUM (accumulating across groups) - this both
           reduces over the partition axis and broadcasts the column
           sums to all 128 partitions. ScalarE folds the 0.75 scale
           while moving PSUM -> SBUF.
  Phase 2: per x row-tile [128, 1024]: VectorE multiply against the
           broadcast column sums into PSUM; reduce along the free dim on
           ScalarE via activation(Copy, accum_out=...) (2 late tiles on
           VectorE to balance the two engines' drain). Results collect
           in SBUF [128, 32], transposed on TensorE so the store is one
           contiguous 16KiB DMA.

(tensor_tensor_reduce would fuse phase 2 into one VectorE op, but that
opcode crashes the device on this HW/NRT path - validated by bisection.)
"""

import numpy as np

B, IN, HID = 4096, 8192, 8192
N_CORES = 8
CHUNK = IN // N_CORES          # 1024 columns per core
SCALE = 1.5 / 2.0              # 0.75
P = 128                        # partitions
W_TILES = HID // P             # 64 weight row-tiles per core
X_TILES = B // P               # 32 x row-tiles per core

# Phase-1 reduction units: each inner list is the DMA-group sizes that are
# pre-reduced on VectorE to ONE [128, 1024] tile before the 2-matmul PSUM
# accumulation. 8-tile units keep the serial PE chain (~3us/unit even at
# the HAM-cold 1.2 GHz clock) well under the unit's ~10-11us DMA window -
# with 4-tile units the chain (~5.6us/group cold) sat right at the group's
# ~5us DMA time and any PE clock-gate jitter stalled wpool recycling and
# drained the DMA ring (observed as 35-125 GB/s bins mid-stream).
W_UNITS = [[4, 4]] * 7 + [[4], [2], [1], [1]]  # 64 tiles, 18 DMAs
X_GROUPS = [1, 1, 2] + [4] * 6 + [2, 1, 1]     # 32 tiles, 12 DMAs
assert sum(sum(u) for u in W_UNITS) == W_TILES and sum(X_GROUPS) == X_TILES

_compiled_nc = None


def _build_nc():
    import concourse.bass as bass
    import concourse.tile as tile
    from concourse import bacc, mybir

    f32 = mybir.dt.float32
    nc = bacc.Bacc(
        "TRN2",
        target_bir_lowering=False,
        debug=False,
        num_devices=N_CORES,
    )

    x_d = nc.dram_tensor("x", [P, X_TILES, CHUNK], f32, kind="ExternalInput")
    w_d = nc.dram_tensor("w", [P, W_TILES, CHUNK], f32, kind="ExternalInput")
    out_d = nc.dram_tensor("out", [B, 1], f32, kind="ExternalOutput")

    with tile.TileContext(nc) as tc:
        with (
            tc.tile_pool(name="wpool", bufs=5) as wpool,
            tc.tile_pool(name="xpool", bufs=5) as xpool,
            tc.tile_pool(name="const", bufs=1) as const,
            tc.tile_pool(name="psum", bufs=1, space="PSUM") as psum_pool,
        ):
            from concourse.masks import make_identity

            ones = const.tile([P, P], f32)
            nc.vector.memset(ones[:], 1.0)
            identity = const.tile([P, P], f32)
            make_identity(nc, identity)

            # Phase 1: column sums of the weight chunk over all 8192 rows.
            psum_bc = psum_pool.tile([P, CHUNK], f32, tag="psum_bc")  # 2 banks
            pos = 0
            for ui, unit in enumerate(W_UNITS):
                wts = []
                for g in unit:
                    wt = wpool.tile([P, 4, CHUNK], f32, tag="wtile")
                    nc.sync.dma_start(wt[:, :g, :], w_d[:, pos : pos + g, :])
                    # in-group tree reduce on VectorE -> wt[:, 0, :]
                    if g == 4:
                        nc.vector.tensor_add(wt[:, 0, :], wt[:, 0, :], wt[:, 1, :])
                        nc.vector.tensor_add(wt[:, 2, :], wt[:, 2, :], wt[:, 3, :])
                        nc.vector.tensor_add(wt[:, 0, :], wt[:, 0, :], wt[:, 2, :])
                    elif g == 2:
                        nc.vector.tensor_add(wt[:, 0, :], wt[:, 0, :], wt[:, 1, :])
                    wts.append(wt)
                    pos += g
                # cross-group reduce to one tile per unit
                for other in wts[1:]:
                    nc.vector.tensor_add(
                        wts[0][:, 0, :], wts[0][:, 0, :], other[:, 0, :]
                    )
                for h in range(2):
                    nc.tensor.matmul(
                        psum_bc[:, h * 512 : (h + 1) * 512],
                        ones[:],
                        wts[0][:, 0, h * 512 : (h + 1) * 512],
                        start=(ui == 0),
                        stop=(ui == len(W_UNITS) - 1),
                    )

            # Broadcast column sums now live in every PSUM partition; move to
            # SBUF on ScalarE (folding in the 0.75 scale) so VectorE stays
            # free for phase 2.
            w_bcast = const.tile([P, CHUNK], f32)
            nc.scalar.mul(w_bcast[:], psum_bc[:], SCALE)

            # Phase 2: multiply + reduce of x tiles against w_bcast.
            # ScalarE activation(Copy, accum_out) does most row-reductions
            # (reads PSUM at its lower base cost); VectorE takes two late
            # tiles to balance the engines' drain after the stream ends.
            DVE_REDUCE = {29, 31}
            s_sbuf = const.tile([P, X_TILES], f32)
            scratch = const.tile([P, CHUNK], f32)
            pos = 0
            for g in X_GROUPS:
                xt = xpool.tile([P, 4, CHUNK], f32, tag="xtile")
                nc.sync.dma_start(xt[:, :g, :], x_d[:, pos : pos + g, :])
                for j in range(g):
                    i = pos + j
                    prod = psum_pool.tile([P, CHUNK], f32, tag="prodps", bufs=2)
                    nc.vector.tensor_mul(prod[:], xt[:, j, :], w_bcast[:])
                    if i in DVE_REDUCE:
                        nc.vector.reduce_sum(
                            s_sbuf[:, i : i + 1], prod[:], axis=mybir.AxisListType.X
                        )
                    else:
                        nc.scalar.activation(
                            scratch[:],
                            prod[:],
                            mybir.ActivationFunctionType.Copy,
                            bias=0.0,
                            scale=1.0,
                            accum_out=s_sbuf[:, i : i + 1],
                        )
                pos += g

            # Transpose s_sbuf [128, 32] -> [32, 128] on TensorE so the store
            # is contiguous 512B runs in DRAM (a [128, 32]-layout store would
            # shatter into 4096 4-byte DMA packets - measured 16us).
            psum_t = psum_pool.tile([X_TILES, P], f32, tag="psum_t")
            nc.tensor.transpose(psum_t[:], s_sbuf[:], identity[:])
            sT = const.tile([X_TILES, P], f32)
            nc.scalar.copy(sT[:], psum_t[:])
            # out[n*128 + p, 0] = sT[n, p]
            out_ap = out_d[:].rearrange("(n p) o -> n (p o)", p=P)
            nc.sync.dma_start(out_ap, sT[:])

    nc.compile()
    return nc


def _get_nc():
    global _compiled_nc
    if _compiled_nc is None:
        _compiled_nc = _build_nc()
    return _compiled_nc


def _shard_inputs(x: np.ndarray, weight: np.ndarray):
    """Column-shard both tensors and repack each shard partition-major
    ([128, tiles, 1024]) so every DMA descriptor covers contiguous DRAM."""
    in_maps = []
    for d in range(N_CORES):
        xc = x[:, d * CHUNK : (d + 1) * CHUNK]
        wc = weight[:, d * CHUNK : (d + 1) * CHUNK]
        xr = np.ascontiguousarray(
            xc.reshape(X_TILES, P, CHUNK).transpose(1, 0, 2)
        )
        wr = np.ascontiguousarray(
            wc.reshape(W_TILES, P, CHUNK).transpose(1, 0, 2)
        )
        in_maps.append({"x": xr, "w": wr})
    return in_maps


def kernel(x: np.ndarray, weight: np.ndarray) -> np.ndarray:
    from concourse.bass_utils import run_bass_kernel_spmd

    x = np.asarray(x, dtype=np.float32)
    weight = np.asarray(weight, dtype=np.float32)
    assert x.shape == (B, IN) and weight.shape == (HID, IN)

    nc = _get_nc()
    in_maps = _shard_inputs(x, weight)
    res = run_bass_kernel_spmd(nc, in_maps, core_ids=list(range(N_CORES)))
    acc = np.zeros((B, 1), dtype=np.float64)
    for d in range(N_CORES):
        acc += res.results[d]["out"].astype(np.float64)
    return acc.astype(np.float32)
